# revision 21
# baseline (speedup 1.0000x reference)
"""Trainium2 Bass kernel for nn_DL_R_sum_MRC (MIMO MRC rate-sum loss).

Math (per batch b, RB i, subcarrier j, user k), derived from reference:
  V[c,t]   : unnormalized complex precoder (from y_pred), per (b, i)
  N2[c]    = sum_t |V[c,t]|^2           (normalization folded into the logs)
  hv[r,c]  = sum_t H_k[t,r] * V[c,t]    (complex, unnormalized)
  HF = hv[:,k], G = hv[:,1-k]
  q_u  = sum_r |HF_r|^2
  u_u  = sum_r conj(HF_r) * G_r
  DEN  = N2_k * (sigma * q_u * N2_kb + |u_u|^2 * P_kb)
  NUM  = DEN + q_u^2 * P_k * N2_kb
  rate = (ln NUM - ln DEN) / ln 2
  loss = -sum rate / (B * 52)

Sharding: pure data-parallel over batch, 8 NeuronCores x 512 batch.
Each core reduces its rates to a [128, NCHUNK] partial-sum tile; host sums.

Engine split (per 128-batch chunk):
  DMA : h1/h2 halves, y, P                  (~21.5 us)
  ACT : f32->bf16 cast + (sc,t,r,e)->(sc,r,te) relayout, vboth build,
        v^2 squares, the two Ln's           (~20 us)
  DVE : products (32 instrs), tree L1, n2 reduce, epilogue (~51 us)
  Pool: tree levels L2..L6 (the te-reduction tail)         (~51 us)

On-chip layouts (batch in partitions, 128 per chunk):
  H  (DMA):   (sc, t, r, e)    sc*128 + t*4 + r*2 + e     [P, 6656] per user
  H' (ACT):   (sc, r, te)      sc*128 + r*64 + 2t + e     bf16
  vboth:      [vneg | vswap]   (c, i, te), vneg=(re,-im), vswap=(im,re)
  pr (k,part): (jr, i, c, te)  jr*1664 + i*128 + c*64 + te  bf16
  hv:         (k, part, j, r, i, c)  k*416+part*208+j*52+r*26+i*2+c  f32
"""

import math
import sys

import numpy as np

sys.path.insert(0, "/opt/trn_rl_repo")

B_FULL = 4096
N_CORES = 8
NB = B_FULL // N_CORES  # 512 batch per core
P = 128                 # partitions per chunk
NCHUNK = NB // P        # 4 chunks
SIGMA = 0.1
NRB = 13
NSC = 52

H_FREE = NSC * 32 * 2 * 2   # 6656
Y_FREE = 64 * NRB * 2       # 1664
P_FREE = NRB * 2            # 26: (i, c)

_TRACE = {"on": False, "result": None}
_CFG = {"pool_tree": True}  # False: run tree levels L2..L6 on DVE (debug)


def _ap(x, off, dims):
    """View of tile/dram AP `x` at element offset `off` with free dims [[step, count], ...]."""
    import concourse.bass as bass

    return bass.AP(tensor=x.tensor, offset=x.offset + off, ap=[list(x.ap[0])] + dims)


def _build(nc, repeat=1, parts="all"):
    from contextlib import ExitStack

    import concourse.tile as tile
    from concourse import mybir

    f32 = mybir.dt.float32
    bf16 = mybir.dt.bfloat16
    Alu = mybir.AluOpType
    Act = mybir.ActivationFunctionType
    Ax = mybir.AxisListType

    h1d = nc.dram_tensor("h1", [NB, H_FREE], f32, kind="ExternalInput").ap()
    h2d = nc.dram_tensor("h2", [NB, H_FREE], f32, kind="ExternalInput").ap()
    yd = nc.dram_tensor("yp", [NB, Y_FREE], f32, kind="ExternalInput").ap()
    pd = nc.dram_tensor("pm", [NB, P_FREE], f32, kind="ExternalInput").ap()
    outd = nc.dram_tensor("partial", [P, NCHUNK], f32, kind="ExternalOutput").ap()

    with tile.TileContext(nc) as tc, ExitStack() as ctx:
        hpool = ctx.enter_context(tc.tile_pool(name="hpool", bufs=2))
        hrpool = ctx.enter_context(tc.tile_pool(name="hrpool", bufs=2))
        vpool = ctx.enter_context(tc.tile_pool(name="vpool", bufs=2))
        sqpool = ctx.enter_context(tc.tile_pool(name="sqpool", bufs=1))
        prpool = ctx.enter_context(tc.tile_pool(name="prpool", bufs=1))
        l1pool = ctx.enter_context(tc.tile_pool(name="l1pool", bufs=2))
        ltpool = ctx.enter_context(tc.tile_pool(name="ltpool", bufs=1))
        hvpool = ctx.enter_context(tc.tile_pool(name="hvpool", bufs=2))
        epool = ctx.enter_context(tc.tile_pool(name="epool", bufs=2))
        persist = ctx.enter_context(tc.tile_pool(name="persist", bufs=1))

        racc = persist.tile([P, NCHUNK], f32)

        def epilogue(hv, bkE, bkbE, pkE, pkbE, ch):
            # both users batched via the k-stride trick:
            # hf(k) = hv[k, :, :, :, :, c=k]  -> k-stride 417
            # g(k)  = hv[k, :, :, :, :, c=1-k] -> offset 1, k-stride 415
            kji = [[417, 2], [26, 8], [2, 13]]   # (k, jr, i) of hf
            gji = [[415, 2], [26, 8], [2, 13]]   # (k, jr, i) of g
            t1o = [[104, 2], [13, 8], [1, 13]]   # t1/t2 (k, jr, i) layout
            rin0 = [[104, 2], [26, 4], [1, 13]]  # (k, j, r=0, i)
            rout = [[52, 2], [13, 4], [1, 13]]   # (k, j, i)

            t1 = epool.tile([P, 208], f32, tag="t1")
            t2 = epool.tile([P, 208], f32, tag="t2")
            qu = epool.tile([P, 104], f32, tag="qu")
            ure = epool.tile([P, 104], f32, tag="ure")
            uim = epool.tile([P, 104], f32, tag="uim")

            hf_re, hf_im = _ap(hv, 0, kji), _ap(hv, 208, kji)
            g_re, g_im = _ap(hv, 1, gji), _ap(hv, 209, gji)
            for out, a0, b0, a1, b1, op in (
                (qu, hf_re, hf_re, hf_im, hf_im, Alu.add),
                (ure, hf_re, g_re, hf_im, g_im, Alu.add),
                (uim, hf_re, g_im, hf_im, g_re, Alu.subtract),
            ):
                nc.vector.tensor_mul(_ap(t1, 0, t1o), a0, b0)
                nc.vector.tensor_mul(_ap(t2, 0, t1o), a1, b1)
                nc.vector.tensor_tensor(out=_ap(t1, 0, t1o), in0=_ap(t1, 0, t1o),
                                        in1=_ap(t2, 0, t1o), op=op)
                nc.vector.tensor_add(_ap(out, 0, rout),
                                     _ap(t1, 0, rin0), _ap(t1, 13, rin0))

            uu2 = epool.tile([P, 104], f32, tag="uu2")
            nc.vector.tensor_mul(ure, ure, ure)
            nc.vector.tensor_mul(uim, uim, uim)
            nc.vector.tensor_add(uu2, ure, uim)

            den = epool.tile([P, 104], f32, tag="den")
            num = epool.tile([P, 104], f32, tag="num")
            # den = bk * (sigma*qu*bkb + uu2*pkb)
            nc.vector.scalar_tensor_tensor(
                out=den, in0=qu, scalar=SIGMA, in1=bkbE,
                op0=Alu.mult, op1=Alu.mult)
            nc.vector.tensor_mul(num, uu2, pkbE)
            nc.vector.tensor_add(den, den, num)
            nc.vector.tensor_mul(den, den, bkE)
            # num = den + qu^2 * pk * bkb
            nc.vector.tensor_mul(num, qu, qu)
            nc.vector.tensor_mul(num, num, pkE)
            nc.vector.tensor_mul(num, num, bkbE)
            nc.vector.tensor_add(num, num, den)

            nc.scalar.activation(den, den, Act.Ln)
            nc.scalar.activation(num, num, Act.Ln)
            nc.vector.tensor_sub(num, num, den)
            nc.vector.tensor_reduce(
                out=_ap(racc, ch, [[1, 1]]),
                in_=num, axis=Ax.X, op=Alu.add)

        pending = None
        for ch in [c for _ in range(repeat) for c in range(NCHUNK)]:
            b0 = ch * P
            bsl = slice(b0, b0 + P)

            # h1 first so the products pipeline (DMA h1 -> ACT cast -> DVE
            # products -> Pool tree) starts as early as possible
            h1p = hrpool.tile([P, H_FREE], bf16, tag="h1p")
            h2p = hrpool.tile([P, H_FREE], bf16, tag="h2p")

            def load_h(hd, hp):
                for half in range(2):
                    off = half * 26 * 128
                    hraw = hpool.tile([P, H_FREE // 2], f32, tag="hraw")
                    nc.sync.dma_start(out=hraw, in_=hd[bsl, off:off + 3328])
                    for r in range(2):
                        nc.scalar.copy(
                            _ap(hp, off + r * 64, [[128, 26], [2, 32], [1, 2]]),
                            _ap(hraw, r * 2, [[128, 26], [4, 32], [1, 2]]),
                        )

            yt = vpool.tile([P, Y_FREE], f32, tag="yt")
            nc.sync.dma_start(out=yt, in_=yd[bsl, :])
            pt = vpool.tile([P, P_FREE], f32, tag="pt")
            nc.sync.dma_start(out=pt, in_=pd[bsl, :])
            load_h(h1d, h1p)
            load_h(h2d, h2p)

            # vboth = [vneg | vswap], (c, i, te) each, built from yt (c,t,i,e)
            vboth = vpool.tile([P, 2 * 1664], bf16, tag="vboth")
            for c in range(2):
                # vneg: (re, im) pairs, im negated below
                nc.scalar.copy(
                    _ap(vboth, c * 832, [[64, 13], [2, 32], [1, 2]]),
                    _ap(yt, c * 832, [[2, 13], [26, 32], [1, 2]]),
                )
                # vswap slot0 = im, slot1 = re
                nc.scalar.copy(
                    _ap(vboth, 1664 + c * 832, [[64, 13], [2, 32]]),
                    _ap(yt, c * 832 + 1, [[2, 13], [26, 32]]),
                )
                nc.scalar.copy(
                    _ap(vboth, 1664 + c * 832 + 1, [[64, 13], [2, 32]]),
                    _ap(yt, c * 832, [[2, 13], [26, 32]]),
                )
            ng = _ap(vboth, 1, [[64, 26], [2, 32]])
            nc.scalar.mul(ng, ng, -1.0)

            # N2[c, i] = sum_te v^2 (squares of vneg part: signs cancel)
            ysq = sqpool.tile([P, 1664], f32, tag="ysq")
            nc.scalar.square(ysq, _ap(vboth, 0, [[1, 1664]]))
            n2 = epool.tile([P, 26], f32, tag="n2")
            nc.vector.tensor_reduce(
                out=_ap(n2, 0, [[1, 26]]),
                in_=_ap(ysq, 0, [[64, 26], [1, 64]]),
                axis=Ax.X, op=Alu.add)
            # expanded (k, j, i) broadcast tables for the epilogue chain:
            # bkE = N2[c=k], bkbE = N2[c=1-k], pkE = P[i, c=k], pkbE = P[i, c=1-k]
            kjiE = [[52, 2], [13, 4], [1, 13]]
            bkE = epool.tile([P, 104], f32, tag="bkE")
            nc.scalar.copy(_ap(bkE, 0, kjiE), _ap(n2, 0, [[13, 2], [0, 4], [1, 13]]))
            bkbE = epool.tile([P, 104], f32, tag="bkbE")
            nc.scalar.copy(_ap(bkbE, 0, [[13, 4], [1, 13]]), _ap(n2, 13, [[0, 4], [1, 13]]))
            nc.scalar.copy(_ap(bkbE, 52, [[13, 4], [1, 13]]), _ap(n2, 0, [[0, 4], [1, 13]]))
            pkE = epool.tile([P, 104], f32, tag="pkE")
            nc.scalar.copy(_ap(pkE, 0, kjiE), _ap(pt, 0, [[1, 2], [0, 4], [2, 13]]))
            pkbE = epool.tile([P, 104], f32, tag="pkbE")
            nc.scalar.copy(_ap(pkbE, 0, [[13, 4], [1, 13]]), _ap(pt, 1, [[0, 4], [2, 13]]))
            nc.scalar.copy(_ap(pkbE, 52, [[13, 4], [1, 13]]), _ap(pt, 0, [[0, 4], [2, 13]]))

            if parts == "dmaonly":
                nc.vector.tensor_copy(_ap(racc, ch, [[1, 1]]),
                                      _ap(n2, 0, [[1, 1]]))
                continue

            # ---- products + te-reduction tree ----
            # hv (k, part, j, r, i, c) f32
            hv = hvpool.tile([P, 832], f32, tag="hv")
            for u in range(4):
                    k, part = divmod(u, 2)
                    hp = h1p if k == 0 else h2p
                    pr = prpool.tile([P, 13312], bf16, tag="pr")
                    for jr in range(8):
                        nc.vector.tensor_mul(
                            _ap(pr, jr * 1664, [[128, 13], [64, 2], [1, 64]]),
                            _ap(hp, jr * 64, [[512, 13], [0, 2], [1, 64]]),
                            _ap(vboth, part * 1664, [[64, 13], [832, 2], [1, 64]]),
                        )
                    # L1 on DVE (bf16 2x), L2..L6 on Pool
                    pl1 = l1pool.tile([P, 6656], bf16, tag="pl1")
                    nc.vector.tensor_add(
                        _ap(pl1, 0, [[32, 208], [1, 32]]),
                        _ap(pr, 0, [[64, 208], [1, 32]]),
                        _ap(pr, 32, [[64, 208], [1, 32]]))
                    if parts == "prodonly":
                        continue
                    pl2 = ltpool.tile([P, 3328], bf16, tag="pl2")
                    nc.gpsimd.tensor_add(
                        _ap(pl2, 0, [[16, 208], [1, 16]]),
                        _ap(pl1, 0, [[32, 208], [1, 16]]),
                        _ap(pl1, 16, [[32, 208], [1, 16]]))
                    pl3 = ltpool.tile([P, 1664], bf16, tag="pl3")
                    nc.gpsimd.tensor_add(
                        _ap(pl3, 0, [[8, 208], [1, 8]]),
                        _ap(pl2, 0, [[16, 208], [1, 8]]),
                        _ap(pl2, 8, [[16, 208], [1, 8]]))
                    pl4 = ltpool.tile([P, 832], bf16, tag="pl4")
                    nc.gpsimd.tensor_add(
                        _ap(pl4, 0, [[4, 208], [1, 4]]),
                        _ap(pl3, 0, [[8, 208], [1, 4]]),
                        _ap(pl3, 4, [[8, 208], [1, 4]]))
                    pl5 = ltpool.tile([P, 416], bf16, tag="pl5")
                    nc.gpsimd.tensor_add(
                        _ap(pl5, 0, [[2, 208], [1, 2]]),
                        _ap(pl4, 0, [[4, 208], [1, 2]]),
                        _ap(pl4, 2, [[4, 208], [1, 2]]))
                    nc.gpsimd.tensor_add(
                        _ap(hv, u * 208, [[1, 208]]),
                        _ap(pl5, 0, [[2, 208]]),
                        _ap(pl5, 1, [[2, 208]]))

            if parts == "prodonly":
                nc.vector.tensor_copy(_ap(racc, ch, [[1, 1]]),
                                      _ap(hv, 0, [[1, 1]]))
                continue

            # epilogue of the PREVIOUS chunk goes here, so it never waits on
            # this chunk's Pool tree chain (software pipelining)
            if pending is not None:
                epilogue(*pending)
            pending = (hv, bkE, bkbE, pkE, pkbE, ch)

        if pending is not None:
            epilogue(*pending)

        nc.sync.dma_start(out=outd, in_=racc)

    return nc


def _make_program(repeat=1):
    from concourse import bacc

    nc = bacc.Bacc("TRN2", target_bir_lowering=False, debug=False,
                   num_devices=N_CORES)
    _build(nc, repeat=repeat)
    nc.compile()
    return nc


def kernel(H_dl_RB_1, H_dl_RB_2, P_marix, y_pred):
    from concourse.bass_utils import run_bass_kernel_spmd

    h1 = np.ascontiguousarray(np.asarray(H_dl_RB_1, dtype=np.float32)).reshape(B_FULL, H_FREE)
    h2 = np.ascontiguousarray(np.asarray(H_dl_RB_2, dtype=np.float32)).reshape(B_FULL, H_FREE)
    yp = np.ascontiguousarray(np.asarray(y_pred, dtype=np.float32)).reshape(B_FULL, Y_FREE)
    pm = np.ascontiguousarray(np.asarray(P_marix, dtype=np.float32)).reshape(B_FULL, P_FREE)

    nc = _make_program()
    in_maps = []
    for c in range(N_CORES):
        s = slice(c * NB, (c + 1) * NB)
        in_maps.append({"h1": h1[s], "h2": h2[s], "yp": yp[s], "pm": pm[s]})

    res = run_bass_kernel_spmd(nc, in_maps, list(range(N_CORES)),
                               trace=_TRACE["on"])
    _TRACE["result"] = res
    total = np.float64(0.0)
    for r in res.results:
        total += np.float64(r["partial"].astype(np.float64).sum())
    loss = -total / (math.log(2.0) * B_FULL * NSC)
    return np.float32(loss)


# revision 30
# speedup vs baseline: 2.0141x; 2.0141x over previous
"""Trainium2 Bass kernel for nn_DL_R_sum_MRC (MIMO MRC rate-sum loss).

Math (per batch b, RB i, subcarrier j, user k), derived from reference:
  V[c,t]   : unnormalized complex precoder (from y_pred), per (b, i)
  N2[c]    = sum_t |V[c,t]|^2           (normalization folded into the logs)
  hv[r,c]  = sum_t H_k[t,r] * V[c,t]    (complex, unnormalized)
  HF = hv[:,k], G = hv[:,1-k]
  q_u  = sum_r |HF_r|^2
  u_u  = sum_r conj(HF_r) * G_r
  DEN  = N2_k * (sigma * q_u * N2_kb + |u_u|^2 * P_kb)
  NUM  = DEN + q_u^2 * P_k * N2_kb
  rate = (ln NUM - ln DEN) / ln 2
  loss = -sum rate / (B * 52)

Sharding: pure data-parallel over batch, 8 NeuronCores x 512 batch.
Each core reduces its rates to a [128, NCHUNK] partial-sum tile; host sums.

Engine split (per 128-batch chunk):
  DMA : h1/h2 halves, y, P                  (~21.5 us)
  ACT : f32->bf16 cast + (sc,t,r,e)->(sc,r,te) relayout, vboth build,
        v^2 squares, the two Ln's           (~20 us)
  DVE : products (32 instrs), tree L1, n2 reduce, epilogue (~51 us)
  Pool: tree levels L2..L6 (the te-reduction tail)         (~51 us)

On-chip layouts (batch in partitions, 128 per chunk):
  H  (DMA):   (sc, t, r, e)    sc*128 + t*4 + r*2 + e     [P, 6656] per user
  H' (ACT):   (sc, r, te)      sc*128 + r*64 + 2t + e     bf16
  vboth:      [vneg | vswap]   (c, i, te), vneg=(re,-im), vswap=(im,re)
  pr (k,part): (jr, i, c, te)  jr*1664 + i*128 + c*64 + te  bf16
  hv:         (k, part, j, r, i, c)  k*416+part*208+j*52+r*26+i*2+c  f32
"""

import math
import sys

import numpy as np

sys.path.insert(0, "/opt/trn_rl_repo")

B_FULL = 4096
N_CORES = 8
NB = B_FULL // N_CORES  # 512 batch per core
P = 128                 # partitions per chunk
NCHUNK = NB // P        # 4 chunks
SIGMA = 0.1
NRB = 13
NSC = 52

H_FREE = NSC * 32 * 2 * 2   # 6656
Y_FREE = 64 * NRB * 2       # 1664
P_FREE = NRB * 2            # 26: (i, c)

_TRACE = {"on": False, "result": None}
_CFG = {"pool_tree": True}  # False: run tree levels L2..L6 on DVE (debug)


def _ap(x, off, dims):
    """View of tile/dram AP `x` at element offset `off` with free dims [[step, count], ...]."""
    import concourse.bass as bass

    return bass.AP(tensor=x.tensor, offset=x.offset + off, ap=[list(x.ap[0])] + dims)


def _build(nc, repeat=1, parts="all"):
    from contextlib import ExitStack

    import concourse.tile as tile
    from concourse import mybir

    f32 = mybir.dt.float32
    bf16 = mybir.dt.bfloat16
    Alu = mybir.AluOpType
    Act = mybir.ActivationFunctionType
    Ax = mybir.AxisListType

    h1d = nc.dram_tensor("h1", [NB, H_FREE], f32, kind="ExternalInput").ap()
    h2d = nc.dram_tensor("h2", [NB, H_FREE], f32, kind="ExternalInput").ap()
    yd = nc.dram_tensor("yp", [NB, Y_FREE], f32, kind="ExternalInput").ap()
    pd = nc.dram_tensor("pm", [NB, P_FREE], f32, kind="ExternalInput").ap()
    outd = nc.dram_tensor("partial", [P, NCHUNK], f32, kind="ExternalOutput").ap()

    with tile.TileContext(nc) as tc, ExitStack() as ctx:
        hpool = ctx.enter_context(tc.tile_pool(name="hpool", bufs=2))
        hrpool = ctx.enter_context(tc.tile_pool(name="hrpool", bufs=2))
        vpool = ctx.enter_context(tc.tile_pool(name="vpool", bufs=2))
        sqpool = ctx.enter_context(tc.tile_pool(name="sqpool", bufs=1))
        prpool = ctx.enter_context(tc.tile_pool(name="prpool", bufs=1))
        l1pool = ctx.enter_context(tc.tile_pool(name="l1pool", bufs=2))
        ltpool = ctx.enter_context(tc.tile_pool(name="ltpool", bufs=1))
        hvpool = ctx.enter_context(tc.tile_pool(name="hvpool", bufs=2))
        epool = ctx.enter_context(tc.tile_pool(name="epool", bufs=2))
        persist = ctx.enter_context(tc.tile_pool(name="persist", bufs=1))

        racc = persist.tile([P, NCHUNK], f32)

        def epilogue(hv, bkE, bkbE, pkE, pkbE, ch):
            # both users batched via the k-stride trick:
            # hf(k) = hv[k, :, :, :, :, c=k]  -> k-stride 417
            # g(k)  = hv[k, :, :, :, :, c=1-k] -> offset 1, k-stride 415
            kji = [[417, 2], [26, 8], [2, 13]]   # (k, jr, i) of hf
            gji = [[415, 2], [26, 8], [2, 13]]   # (k, jr, i) of g
            t1o = [[104, 2], [13, 8], [1, 13]]   # t1/t2 (k, jr, i) layout
            rin0 = [[104, 2], [26, 4], [1, 13]]  # (k, j, r=0, i)
            rout = [[52, 2], [13, 4], [1, 13]]   # (k, j, i)

            t1 = epool.tile([P, 208], f32, tag="t1")
            t2 = epool.tile([P, 208], f32, tag="t2")
            qu = epool.tile([P, 104], f32, tag="qu")
            ure = epool.tile([P, 104], f32, tag="ure")
            uim = epool.tile([P, 104], f32, tag="uim")

            hf_re, hf_im = _ap(hv, 0, kji), _ap(hv, 208, kji)
            g_re, g_im = _ap(hv, 1, gji), _ap(hv, 209, gji)
            for out, a0, b0, a1, b1, op in (
                (qu, hf_re, hf_re, hf_im, hf_im, Alu.add),
                (ure, hf_re, g_re, hf_im, g_im, Alu.add),
                (uim, hf_re, g_im, hf_im, g_re, Alu.subtract),
            ):
                nc.vector.tensor_mul(_ap(t1, 0, t1o), a0, b0)
                nc.vector.tensor_mul(_ap(t2, 0, t1o), a1, b1)
                nc.vector.tensor_tensor(out=_ap(t1, 0, t1o), in0=_ap(t1, 0, t1o),
                                        in1=_ap(t2, 0, t1o), op=op)
                nc.vector.tensor_add(_ap(out, 0, rout),
                                     _ap(t1, 0, rin0), _ap(t1, 13, rin0))

            uu2 = epool.tile([P, 104], f32, tag="uu2")
            nc.vector.tensor_mul(ure, ure, ure)
            nc.vector.tensor_mul(uim, uim, uim)
            nc.vector.tensor_add(uu2, ure, uim)

            den = epool.tile([P, 104], f32, tag="den")
            num = epool.tile([P, 104], f32, tag="num")
            # den = bk * (sigma*qu*bkb + uu2*pkb)
            nc.vector.scalar_tensor_tensor(
                out=den, in0=qu, scalar=SIGMA, in1=bkbE,
                op0=Alu.mult, op1=Alu.mult)
            nc.vector.tensor_mul(num, uu2, pkbE)
            nc.vector.tensor_add(den, den, num)
            nc.vector.tensor_mul(den, den, bkE)
            # num = den + qu^2 * pk * bkb
            nc.vector.tensor_mul(num, qu, qu)
            nc.vector.tensor_mul(num, num, pkE)
            nc.vector.tensor_mul(num, num, bkbE)
            nc.vector.tensor_add(num, num, den)

            nc.scalar.activation(den, den, Act.Ln)
            nc.scalar.activation(num, num, Act.Ln)
            nc.vector.tensor_sub(num, num, den)
            nc.vector.tensor_reduce(
                out=_ap(racc, ch, [[1, 1]]),
                in_=num, axis=Ax.X, op=Alu.add)

        pending = None
        for ch in [c for _ in range(repeat) for c in range(NCHUNK)]:
            b0 = ch * P
            bsl = slice(b0, b0 + P)

            # h1 first so the products pipeline (DMA h1 -> ACT cast -> DVE
            # products -> Pool tree) starts as early as possible
            h1p = hrpool.tile([P, H_FREE], bf16, tag="h1p")
            h2p = hrpool.tile([P, H_FREE], bf16, tag="h2p")

            def load_h(hd, hp):
                for half in range(2):
                    off = half * 26 * 128
                    hraw = hpool.tile([P, H_FREE // 2], f32, tag="hraw")
                    nc.sync.dma_start(out=hraw, in_=hd[bsl, off:off + 3328])
                    for r in range(2):
                        nc.scalar.copy(
                            _ap(hp, off + r * 64, [[128, 26], [2, 32], [1, 2]]),
                            _ap(hraw, r * 2, [[128, 26], [4, 32], [1, 2]]),
                        )

            yt = vpool.tile([P, Y_FREE], f32, tag="yt")
            nc.sync.dma_start(out=yt, in_=yd[bsl, :])
            pt = vpool.tile([P, P_FREE], f32, tag="pt")
            nc.sync.dma_start(out=pt, in_=pd[bsl, :])

            # vboth = [vneg | vswap], (c, i, te) each, built from yt (c,t,i,e)
            # (emitted before the h casts so ACT finishes it first and the
            # products pipeline can start as soon as h1 is cast)
            vboth = vpool.tile([P, 2 * 1664], bf16, tag="vboth")
            for c in range(2):
                # vneg: (re, im) pairs, im negated below
                nc.scalar.copy(
                    _ap(vboth, c * 832, [[64, 13], [2, 32], [1, 2]]),
                    _ap(yt, c * 832, [[2, 13], [26, 32], [1, 2]]),
                )
                # vswap slot0 = im, slot1 = re
                nc.scalar.copy(
                    _ap(vboth, 1664 + c * 832, [[64, 13], [2, 32]]),
                    _ap(yt, c * 832 + 1, [[2, 13], [26, 32]]),
                )
                nc.scalar.copy(
                    _ap(vboth, 1664 + c * 832 + 1, [[64, 13], [2, 32]]),
                    _ap(yt, c * 832, [[2, 13], [26, 32]]),
                )
            ng = _ap(vboth, 1, [[64, 26], [2, 32]])
            nc.scalar.mul(ng, ng, -1.0)

            load_h(h1d, h1p)
            load_h(h2d, h2p)

            if parts == "dmaonly":
                nc.vector.tensor_copy(_ap(racc, ch, [[1, 1]]),
                                      _ap(yt, 0, [[1, 1]]))
                continue

            # ---- products + te-reduction tree ----
            # hv (k, part, j, r, i, c) f32
            hv = hvpool.tile([P, 832], f32, tag="hv")
            for u in range(4):
                    k, part = divmod(u, 2)
                    hp = h1p if k == 0 else h2p
                    pr = prpool.tile([P, 13312], bf16, tag="pr")
                    for jr in range(8):
                        nc.vector.tensor_mul(
                            _ap(pr, jr * 1664, [[128, 13], [64, 2], [1, 64]]),
                            _ap(hp, jr * 64, [[512, 13], [0, 2], [1, 64]]),
                            _ap(vboth, part * 1664, [[64, 13], [832, 2], [1, 64]]),
                        )
                    # L1 on DVE (bf16 2x), L2..L6 on Pool
                    pl1 = l1pool.tile([P, 6656], bf16, tag="pl1")
                    nc.vector.tensor_add(
                        _ap(pl1, 0, [[32, 208], [1, 32]]),
                        _ap(pr, 0, [[64, 208], [1, 32]]),
                        _ap(pr, 32, [[64, 208], [1, 32]]))
                    if parts == "prodonly":
                        continue
                    # final unit of the final chunk: run the tree on DVE — no
                    # later products exist to block in DVE's queue, and Pool's
                    # ~13us tail would otherwise gate the last epilogue (drain)
                    te = nc.vector if (ch == NCHUNK - 1 and u == 3) else nc.gpsimd
                    pl2 = ltpool.tile([P, 3328], bf16, tag="pl2")
                    te.tensor_add(
                        _ap(pl2, 0, [[16, 208], [1, 16]]),
                        _ap(pl1, 0, [[32, 208], [1, 16]]),
                        _ap(pl1, 16, [[32, 208], [1, 16]]))
                    pl3 = ltpool.tile([P, 1664], bf16, tag="pl3")
                    te.tensor_add(
                        _ap(pl3, 0, [[8, 208], [1, 8]]),
                        _ap(pl2, 0, [[16, 208], [1, 8]]),
                        _ap(pl2, 8, [[16, 208], [1, 8]]))
                    pl4 = ltpool.tile([P, 832], bf16, tag="pl4")
                    te.tensor_add(
                        _ap(pl4, 0, [[4, 208], [1, 4]]),
                        _ap(pl3, 0, [[8, 208], [1, 4]]),
                        _ap(pl3, 4, [[8, 208], [1, 4]]))
                    pl5 = ltpool.tile([P, 416], bf16, tag="pl5")
                    te.tensor_add(
                        _ap(pl5, 0, [[2, 208], [1, 2]]),
                        _ap(pl4, 0, [[4, 208], [1, 2]]),
                        _ap(pl4, 2, [[4, 208], [1, 2]]))
                    te.tensor_add(
                        _ap(hv, u * 208, [[1, 208]]),
                        _ap(pl5, 0, [[2, 208]]),
                        _ap(pl5, 1, [[2, 208]]))

            if parts == "prodonly":
                nc.vector.tensor_copy(_ap(racc, ch, [[1, 1]]),
                                      _ap(hv, 0, [[1, 1]]))
                continue

            # N2 + epilogue broadcast tables, emitted AFTER the unit loop so
            # the DVE reduce never delays the products in DVE's in-order queue
            # (none of this is consumed until this chunk's epilogue, which runs
            # during the NEXT chunk's section)
            ysq = sqpool.tile([P, 1664], f32, tag="ysq")
            nc.scalar.square(ysq, _ap(vboth, 0, [[1, 1664]]))
            n2 = epool.tile([P, 26], f32, tag="n2")
            nc.vector.tensor_reduce(
                out=_ap(n2, 0, [[1, 26]]),
                in_=_ap(ysq, 0, [[64, 26], [1, 64]]),
                axis=Ax.X, op=Alu.add)
            # bkE = N2[c=k], bkbE = N2[c=1-k], pkE = P[i, c=k], pkbE = P[i, c=1-k]
            kjiE = [[52, 2], [13, 4], [1, 13]]
            bkE = epool.tile([P, 104], f32, tag="bkE")
            nc.scalar.copy(_ap(bkE, 0, kjiE), _ap(n2, 0, [[13, 2], [0, 4], [1, 13]]))
            bkbE = epool.tile([P, 104], f32, tag="bkbE")
            nc.scalar.copy(_ap(bkbE, 0, [[13, 4], [1, 13]]), _ap(n2, 13, [[0, 4], [1, 13]]))
            nc.scalar.copy(_ap(bkbE, 52, [[13, 4], [1, 13]]), _ap(n2, 0, [[0, 4], [1, 13]]))
            pkE = epool.tile([P, 104], f32, tag="pkE")
            nc.scalar.copy(_ap(pkE, 0, kjiE), _ap(pt, 0, [[1, 2], [0, 4], [2, 13]]))
            pkbE = epool.tile([P, 104], f32, tag="pkbE")
            nc.scalar.copy(_ap(pkbE, 0, [[13, 4], [1, 13]]), _ap(pt, 1, [[0, 4], [2, 13]]))
            nc.scalar.copy(_ap(pkbE, 52, [[13, 4], [1, 13]]), _ap(pt, 0, [[0, 4], [2, 13]]))

            # epilogue of the PREVIOUS chunk goes here, so it never waits on
            # this chunk's Pool tree chain (software pipelining)
            if pending is not None:
                epilogue(*pending)
            pending = (hv, bkE, bkbE, pkE, pkbE, ch)

        if pending is not None:
            epilogue(*pending)

        nc.sync.dma_start(out=outd, in_=racc)

    return nc


def _make_program(repeat=1):
    from concourse import bacc

    nc = bacc.Bacc("TRN2", target_bir_lowering=False, debug=False,
                   num_devices=N_CORES)
    _build(nc, repeat=repeat)
    nc.compile()
    return nc


def kernel(H_dl_RB_1, H_dl_RB_2, P_marix, y_pred):
    from concourse.bass_utils import run_bass_kernel_spmd

    h1 = np.ascontiguousarray(np.asarray(H_dl_RB_1, dtype=np.float32)).reshape(B_FULL, H_FREE)
    h2 = np.ascontiguousarray(np.asarray(H_dl_RB_2, dtype=np.float32)).reshape(B_FULL, H_FREE)
    yp = np.ascontiguousarray(np.asarray(y_pred, dtype=np.float32)).reshape(B_FULL, Y_FREE)
    pm = np.ascontiguousarray(np.asarray(P_marix, dtype=np.float32)).reshape(B_FULL, P_FREE)

    nc = _make_program()
    in_maps = []
    for c in range(N_CORES):
        s = slice(c * NB, (c + 1) * NB)
        in_maps.append({"h1": h1[s], "h2": h2[s], "yp": yp[s], "pm": pm[s]})

    res = run_bass_kernel_spmd(nc, in_maps, list(range(N_CORES)),
                               trace=_TRACE["on"])
    _TRACE["result"] = res
    total = np.float64(0.0)
    for r in res.results:
        total += np.float64(r["partial"].astype(np.float64).sum())
    loss = -total / (math.log(2.0) * B_FULL * NSC)
    return np.float32(loss)
